# revision 1
# baseline (speedup 1.0000x reference)
"""Multi-head self-attention (RoPE, causal) Trainium2 Bass kernel.

Sharding: 8 cores = 4 batches x 2 head-groups (8 heads each).
Per core the device kernel computes, for its batch b and head-group g:
    q/k/v = x_b @ W*[:, g] (+bias), RoPE on q/k, causal softmax attention,
    partial out-projection y @ Wo[g]  -> [2048, 1024] (f32).
Host sums the two head-group partials per batch and adds bo.

RoPE runs in a per-head permuted basis (evens then odds) prepared on the
host by permuting Wq/Wk columns: rotate-half becomes a swap of contiguous
32-partition halves (two SBUF->SBUF DMAs) with the signs folded into the
sin table, so no PE permutation matmul is needed.  Scores are invariant
to the shared q/k basis permutation.

Device layouts (per core):
    xT   [1024, 2048] bf16   x_b transposed (host-prepped: sharding step)
    qT'  [128, 4, 2048] bf16 roped queries, head-pair dims on partitions
    kT'  [128, 8, 2048] bf16 roped keys, per head zero-padded to 128 rows
                             (full-height lhsT keeps fast weight load on)
    v    [128, 16kb, 8h, 65] bf16, col 64 = ones (softmax denominator)
    att  [128, 17408] bf16   exp(scores^T) per head, causal-trapezoid packed
    y    [128, 16qb, 128] x4 per-head-pair attention outputs
    yT   [128, 4, 2048] bf16 transposed y for the out-projection
"""

import os
import sys

import numpy as np

for _p in ("/opt/trn_rl_repo", "/root/.axon_site/_ro/trn_rl_repo"):
    if os.path.isdir(_p) and _p not in sys.path:
        sys.path.append(_p)

import ml_dtypes  # noqa: E402

BF16 = ml_dtypes.bfloat16

B, S, D_MODEL = 4, 2048, 1024
N_HEADS, HEAD_DIM = 16, 64
N_CORES = 8
HG = 2                      # head groups
HPC = N_HEADS // HG         # heads per core = 8
DL = HPC * HEAD_DIM         # local dims per core = 512
SCALE = HEAD_DIM ** -0.5
P = 128
KC = D_MODEL // P           # k chunks in projections = 8
MB = DL // P                # m blocks (head pairs) = 4
NKB = S // P                # 128-row blocks of sequence = 16
QK_PSUM_W = 1536            # scores psum tile width (3 banks)
HH = HEAD_DIM // 2          # 32

# packed causal-trapezoid offsets: att row-block ck covers q in [128*ck, S)
ATT_OFF = [0] * (NKB + 1)
for _ck in range(NKB):
    ATT_OFF[_ck + 1] = ATT_OFF[_ck] + (S - P * _ck)
ATT_TOT = ATT_OFF[NKB]      # 17408

_CACHE = {}
STAGE_OF = {}


def _tag(inst, stage):
    try:
        STAGE_OF[str(inst.ins.name)] = stage
    except Exception:
        pass
    return inst


def _build_bass():
    import concourse.tile as tile
    from concourse import bacc, mybir

    dt = mybir.dt
    nc = bacc.Bacc("TRN2", target_bir_lowering=False, debug=False)

    def din(name, shape, d=dt.bfloat16):
        return nc.dram_tensor(name, shape, d, kind="ExternalInput").ap()

    xT_d = din("xT", [D_MODEL, S])
    wq_d = din("wq", [D_MODEL, DL])
    wk_d = din("wk", [D_MODEL, DL])
    wv_d = din("wv", [D_MODEL, DL])
    wo_d = din("wo", [DL, D_MODEL])
    bq_d = din("bqT", [P, MB], dt.float32)
    bk_d = din("bkT", [P, MB], dt.float32)
    bv_d = din("bv", [1, DL])
    cos_d = din("cosT", [P, S])
    sin_d = din("sinT", [P, S])          # sign-folded (pi-basis)
    tri_d = din("tri", [P, P])
    ident_d = din("ident", [P, P])
    o_d = nc.dram_tensor("o", [S, D_MODEL], dt.float32, kind="ExternalOutput").ap()

    FCopy = mybir.ActivationFunctionType.Copy
    FExp = mybir.ActivationFunctionType.Exp
    NT = S // DL  # 4 sequence tiles of 512

    with tile.TileContext(nc) as tc:
        with (
            tc.tile_pool(name="persist", bufs=1) as persist,
            tc.tile_pool(name="small", bufs=1) as small,
        ):
            qTf = persist.tile([P, MB, S], dt.bfloat16, tag="qTf")
            kTf = persist.tile([P, HPC, S], dt.bfloat16, tag="kTf")
            nc.vector.memset(kTf, 0.0)
            v_sb = persist.tile([P, NKB, HPC, HEAD_DIM + 1], dt.bfloat16, tag="v_sb")
            yT_all = persist.tile([P, MB, S], dt.bfloat16, tag="yT")
            y_mb = [persist.tile([P, NKB, P], dt.bfloat16, tag=f"y_mb{m}",
                                 name=f"y_mb{m}")
                    for m in range(MB)]
            wo_sb = persist.tile([P, MB, D_MODEL], dt.bfloat16)

            tri_sb = small.tile([P, P], dt.bfloat16, tag="tri")
            ident_sb = small.tile([P, P], dt.bfloat16, tag="ident")
            ones_sb = small.tile([1, DL], dt.bfloat16, tag="ones")
            nc.vector.memset(ones_sb, 1.0)
            nc.vector.memset(v_sb[:, :, :, HEAD_DIM:HEAD_DIM + 1], 1.0)

            # ================= Stage B: projections + RoPE =================
            with (
                tc.tile_pool(name="bweights", bufs=1) as bweights,
                tc.tile_pool(name="bstage", bufs=3) as bstage,
                tc.tile_pool(name="proj_ps", bufs=4, space="PSUM") as proj_ps,
            ):
                xT_sb = bweights.tile([P, KC, S], dt.bfloat16, tag="xT")
                w_sbs = {}
                b_sbs = {}
                for nm in ("q", "k", "v"):
                    w_sbs[nm] = bweights.tile([P, KC, DL], dt.bfloat16,
                                              tag=f"w{nm}", name=f"w{nm}")
                cos_sb = bweights.tile([P, S], dt.bfloat16, tag="cos")
                sin_sb = bweights.tile([P, S], dt.bfloat16, tag="sin")
                for nm in ("q", "k"):
                    b_sbs[nm] = small.tile([P, MB], dt.float32,
                                           tag=f"b{nm}", name=f"b{nm}")
                b_sbs["v"] = small.tile([1, DL], dt.bfloat16, tag="bv", name="bv")

                # input DMAs, ordered so the first projection group's data
                # (wq + xT t0 columns) lands first
                def dma_xt(t):
                    for kc in range(KC):
                        nc.sync.dma_start(
                            out=xT_sb[:, kc, t * DL:(t + 1) * DL],
                            in_=xT_d[kc * P:(kc + 1) * P, t * DL:(t + 1) * DL])

                nc.sync.dma_start(out=b_sbs["q"], in_=bq_d)
                nc.sync.dma_start(out=b_sbs["k"], in_=bk_d)
                for kc in range(KC):
                    nc.scalar.dma_start(out=w_sbs["q"][:, kc, :],
                                        in_=wq_d[kc * P:(kc + 1) * P, :])
                dma_xt(0)
                nc.scalar.dma_start(out=cos_sb, in_=cos_d)
                nc.scalar.dma_start(out=sin_sb, in_=sin_d)
                dma_xt(1)
                dma_xt(2)
                for kc in range(KC):
                    nc.scalar.dma_start(out=w_sbs["k"][:, kc, :],
                                        in_=wk_d[kc * P:(kc + 1) * P, :])
                dma_xt(3)
                for kc in range(KC):
                    nc.scalar.dma_start(out=w_sbs["v"][:, kc, :],
                                        in_=wv_d[kc * P:(kc + 1) * P, :])
                nc.scalar.dma_start(out=b_sbs["v"], in_=bv_d)
                nc.scalar.dma_start(out=tri_sb, in_=tri_d)
                nc.scalar.dma_start(out=ident_sb, in_=ident_d)
                nc.scalar.dma_start(
                    out=wo_sb, in_=wo_d.rearrange("(m p) n -> p m n", p=P))

                def emit_v(kb):
                    # v projection: natural [seq, dims] layout + ones cols
                    ps = proj_ps.tile([P, DL], dt.float32, tag="proj",
                                      name="ps_v")
                    for kc in range(KC):
                        _tag(nc.tensor.matmul(
                            ps, lhsT=xT_sb[:, kc, kb * P:(kb + 1) * P],
                            rhs=w_sbs["v"][:, kc, :],
                            start=(kc == 0), stop=False), "proj_v")
                    _tag(nc.tensor.matmul(
                        ps, lhsT=ones_sb[:, :P], rhs=b_sbs["v"],
                        start=False, stop=True), "bias_v")
                    nc.scalar.activation(
                        out=v_sb[:, kb, :, 0:HEAD_DIM],
                        in_=ps.rearrange("p (h d) -> p h d", h=HPC), func=FCopy)

                for nm, t, m in ([("q", t_, m_) for t_ in range(NT)
                                  for m_ in range(MB)] +
                                 [("k", t_, m_) for t_ in range(NT)
                                  for m_ in range(MB)]):
                    if True:
                        if True:
                            w_sb, b_sb = w_sbs[nm], b_sbs[nm]
                            ts = slice(t * DL, (t + 1) * DL)
                            ps = proj_ps.tile([P, DL], dt.float32, tag="proj")
                            for kc in range(KC):
                                _tag(nc.tensor.matmul(
                                    ps, lhsT=w_sb[:, kc, m * P:(m + 1) * P],
                                    rhs=xT_sb[:, kc, ts],
                                    start=(kc == 0), stop=(kc == KC - 1)),
                                    "proj_qk")
                            raw = bstage.tile([P, DL], dt.bfloat16, tag="raw")
                            nc.vector.tensor_scalar(
                                raw, ps, b_sb[:, m:m + 1], None,
                                mybir.AluOpType.add)
                            # rotate-half in the permuted basis: swap the
                            # 32-row halves of each head (sign lives in sinT)
                            qsw = bstage.tile([P, DL], dt.bfloat16, tag="qsw")
                            for hh in range(2):
                                o32 = hh * HEAD_DIM
                                nc.scalar.dma_start(
                                    out=qsw[o32:o32 + HH, :],
                                    in_=raw[o32 + HH:o32 + HEAD_DIM, :])
                                nc.scalar.dma_start(
                                    out=qsw[o32 + HH:o32 + HEAD_DIM, :],
                                    in_=raw[o32:o32 + HH, :])
                            t1 = bstage.tile([P, DL], dt.bfloat16, tag="t1")
                            nc.vector.tensor_mul(t1, raw, cos_sb[:, ts])
                            t2 = bstage.tile([P, DL], dt.bfloat16, tag="t2")
                            nc.gpsimd.tensor_mul(t2, qsw, sin_sb[:, ts])
                            if nm == "q":
                                nc.vector.tensor_add(qTf[:, m, ts], t1, t2)
                            else:
                                for hh in range(2):
                                    po = hh * HEAD_DIM
                                    nc.vector.tensor_add(
                                        kTf[po:po + HEAD_DIM, 2 * m + hh, ts],
                                        t1[po:po + HEAD_DIM, :],
                                        t2[po:po + HEAD_DIM, :])

                for kb in range(NKB):
                    emit_v(kb)


            # ================= Stage C: attention per head =================
            with (
                tc.tile_pool(name="att_pool", bufs=2) as att_pool,
                tc.tile_pool(name="rtile", bufs=4) as rtile,
                tc.tile_pool(name="qk_psA", bufs=1, space="PSUM") as qk_psA,
                tc.tile_pool(name="qk_psB", bufs=1, space="PSUM") as qk_psB,
                tc.tile_pool(name="y_ps", bufs=2, space="PSUM") as y_ps_pool,
                tc.tile_pool(name="tp_ps", bufs=1, space="PSUM") as tp_ps,
            ):
                def emit_c1_steps(h, att):
                    # yields once per ck after emitting QK+exp(+mask)
                    for ck in range(NKB):
                        w = S - ck * P
                        base = ck * P
                        off = ATT_OFF[ck]
                        kh = kTf[:, h, :]
                        qh = qTf[:, h // 2, :]
                        pool, pw = ((qk_psA, 1536) if ck % 2 == 0
                                    else (qk_psB, 1024))
                        for s0 in range(0, w, pw):
                            sw = min(pw, w - s0)
                            ps = pool.tile([P, pw], dt.float32, tag="qk",
                                           name="ps_qk")
                            for u0 in range(0, sw, DL):
                                uw = min(DL, sw - u0)
                                _tag(nc.tensor.matmul(
                                    ps[:, u0:u0 + uw],
                                    lhsT=kh[:, ck * P:(ck + 1) * P],
                                    rhs=qh[:, base + s0 + u0:base + s0 + u0 + uw],
                                    start=True, stop=True), "qk")
                            nc.scalar.activation(
                                out=att[:, off + s0:off + s0 + sw],
                                in_=ps[:, 0:sw], func=FExp, scale=SCALE)
                        nc.gpsimd.tensor_mul(
                            att[:, off:off + P], att[:, off:off + P], tri_sb)
                        yield

                def emit_c2_steps(h, att):
                    # yields once per qb after emitting its AV chain + scale
                    m, po = h // 2, (h % 2) * HEAD_DIM
                    for qb in range(NKB - 1, -1, -1):
                        ys = y_ps_pool.tile([P, HEAD_DIM + 1], dt.float32,
                                            tag="y", name="yp")
                        for ck in range(qb + 1):
                            a0 = ATT_OFF[ck] + (qb - ck) * P
                            _tag(nc.tensor.matmul(
                                ys, lhsT=att[:, a0:a0 + P],
                                rhs=v_sb[:, ck, h, :],
                                start=(ck == 0), stop=(ck == qb)), "av")
                        r = rtile.tile([P, 1], dt.float32, tag="r", name="r")
                        nc.vector.reciprocal(r, ys[:, HEAD_DIM:HEAD_DIM + 1])
                        nc.vector.tensor_scalar(
                            y_mb[m][:, qb, po:po + HEAD_DIM],
                            ys[:, 0:HEAD_DIM], r, None,
                            mybir.AluOpType.mult)
                        yield

                att_tiles = {}
                att_tiles[0] = att_pool.tile([P, ATT_TOT], dt.bfloat16,
                                             tag="att", name="att0")
                for _ in emit_c1_steps(0, att_tiles[0]):
                    pass
                def emit_ytrans(m):
                    for q4 in range(NKB // 4):
                        tp = tp_ps.tile([P, 4 * P], dt.bfloat16, tag="tp",
                                        name="tp")
                        for j in range(4):
                            _tag(nc.tensor.transpose(
                                tp[:, j * P:(j + 1) * P],
                                y_mb[m][:, q4 * 4 + j, :], ident_sb), "ytrans")
                        nc.vector.tensor_copy(
                            out=yT_all[:, m, q4 * 4 * P:(q4 + 1) * 4 * P],
                            in_=tp)

                for h in range(1, HPC):
                    att_tiles[h] = att_pool.tile([P, ATT_TOT], dt.bfloat16,
                                                 tag="att", name=f"att{h}")
                    c1 = emit_c1_steps(h, att_tiles[h])
                    c2 = emit_c2_steps(h - 1, att_tiles[h - 1])
                    alive = True
                    while alive:
                        alive = False
                        if next(c1, "done") != "done":
                            alive = True
                        if next(c2, "done") != "done":
                            alive = True
                    if h >= 2 and h % 2 == 0:
                        emit_ytrans(h // 2 - 1)
                for _ in emit_c2_steps(HPC - 1, att_tiles[HPC - 1]):
                    pass
                emit_ytrans(MB - 1)

            # ============ Stage D: transpose y, out-projection ============
            with (
                tc.tile_pool(name="osb", bufs=3) as osb,
                tc.tile_pool(name="o_ps", bufs=3, space="PSUM") as o_ps_pool,
            ):
                for qb in range(NKB):
                    for t in range(2):
                        ps = o_ps_pool.tile([P, DL], dt.float32, tag="o")
                        for m in range(MB):
                            _tag(nc.tensor.matmul(
                                ps, lhsT=yT_all[:, m, qb * P:(qb + 1) * P],
                                rhs=wo_sb[:, m, t * DL:(t + 1) * DL],
                                start=(m == 0), stop=(m == MB - 1)), "oproj")
                        ob = osb.tile([P, DL], dt.float32, tag="ob")
                        nc.scalar.activation(out=ob, in_=ps, func=FCopy)
                        nc.sync.dma_start(
                            out=o_d[qb * P:(qb + 1) * P, t * DL:(t + 1) * DL],
                            in_=ob)

    nc.compile()
    return nc


def _perm64():
    # pi: permuted-basis index j -> original head dim (evens then odds)
    return np.concatenate([np.arange(0, HEAD_DIM, 2), np.arange(1, HEAD_DIM, 2)])


def _host_tables():
    pos = np.arange(S, dtype=np.float32)
    freq = np.arange(0, HEAD_DIM, 2, dtype=np.float32) / HEAD_DIM
    inv_freq = 1.0 / (10000.0 ** freq)                  # [32]
    ang = np.outer(inv_freq, pos)                       # [32, S]
    cos1 = np.cos(ang)
    sin1 = np.sin(ang)
    # pi-basis per-head tables [64, S]: rows 0..31 evens, 32..63 odds
    cosh = np.concatenate([cos1, cos1], axis=0)
    sinh = np.concatenate([-sin1, sin1], axis=0)        # sign folded in
    cosT = np.tile(cosh, (2, 1))                        # [128, S] head pair
    sinT = np.tile(sinh, (2, 1))
    tri = np.triu(np.ones((P, P), np.float32))          # keep k<=q in [k,q]
    ident = np.eye(P, dtype=np.float32)
    return (cosT.astype(BF16), sinT.astype(BF16),
            tri.astype(BF16), ident.astype(BF16))


def kernel(x, Wq, bq, Wk, bk, Wv, bv, Wo, bo):
    from concourse.bass_utils import run_bass_kernel_spmd

    x = np.asarray(x, np.float32)
    Wq, Wk, Wv, Wo = (np.asarray(a, np.float32) for a in (Wq, Wk, Wv, Wo))
    bq, bk, bv, bo = (np.asarray(a, np.float32) for a in (bq, bk, bv, bo))

    if "nc" not in _CACHE:
        _CACHE["nc"] = _build_bass()
    nc = _CACHE["nc"]

    cosT, sinT, tri, ident = _host_tables()
    consts = {"cosT": cosT, "sinT": sinT, "tri": tri, "ident": ident}

    # pi-basis permutation of q/k projection columns (per head)
    pi = _perm64()
    colperm = np.concatenate([h * HEAD_DIM + pi for h in range(N_HEADS)])
    Wq_p = Wq[:, colperm]
    Wk_p = Wk[:, colperm]
    bq_p = bq[colperm]
    bk_p = bk[colperm]

    xTs = [np.ascontiguousarray(x[b].T).astype(BF16) for b in range(B)]
    in_maps = []
    for c in range(N_CORES):
        b, g = c // HG, c % HG
        sl = slice(g * DL, (g + 1) * DL)
        in_maps.append({
            "xT": xTs[b],
            "wq": np.ascontiguousarray(Wq_p[:, sl]).astype(BF16),
            "wk": np.ascontiguousarray(Wk_p[:, sl]).astype(BF16),
            "wv": np.ascontiguousarray(Wv[:, sl]).astype(BF16),
            "wo": np.ascontiguousarray(Wo[sl, :]).astype(BF16),
            "bqT": np.ascontiguousarray(
                bq_p[sl].reshape(MB, P).T).astype(np.float32),
            "bkT": np.ascontiguousarray(
                bk_p[sl].reshape(MB, P).T).astype(np.float32),
            "bv": bv[sl].reshape(1, DL).astype(BF16),
            **consts,
        })

    res = run_bass_kernel_spmd(nc, in_maps, core_ids=list(range(N_CORES)))
    _CACHE["last_result"] = res
    out = np.empty((B, S, D_MODEL), np.float32)
    for b in range(B):
        out[b] = res.results[HG * b]["o"] + res.results[HG * b + 1]["o"]
    out += bo.astype(np.float32)
    return out



# revision 18
# speedup vs baseline: 1.0562x; 1.0562x over previous
"""Multi-head self-attention (RoPE, causal) Trainium2 Bass kernel.

Sharding: 8 cores = 4 batches x 2 head-groups (8 heads each).
Per core the device kernel computes, for its batch b and head-group g:
    q/k/v = x_b @ W*[:, g] (+bias), RoPE on q/k, causal softmax attention,
    partial out-projection y @ Wo[g]  -> [2048, 1024] (f32).
Host sums the two head-group partials per batch and adds bo.

Pipeline layout (single TileContext, engines overlapped end to end):
    round m = 0..3:  project+RoPE q/k for head pair m (PE+DVE+Pool),
    then attention for heads 2m, 2m+1 (QK on PE -> exp on Act -> AV on PE),
    software-pipelined so the Act engine's exp stream (the serial resource:
    only the Act engine has activation tables) runs concurrently with the
    next group's projections and the previous head's AV matmuls.

The causal mask is folded into the QK PSUM accumulation as an extra matmul
(-1e9*I)^T @ strict_lower_ones on the diagonal block, so exp() gives exact
zeros in the masked region and no separate mask multiply is needed.

RoPE runs in a per-head permuted basis (evens then odds) prepared on the
host by permuting Wq/Wk columns: rotate-half becomes a swap of contiguous
32-partition halves (SBUF->SBUF DMAs) with the signs folded into the sin
table.  Scores are invariant to the shared q/k basis permutation.

Device layouts (per core):
    xT   [128, 8, 2048] bf16  x_b transposed (host-prepped)
    qT   ring[2] of [128, 2048] bf16   roped queries, pair dims on partitions
    kT   ring[2] of [128, 2, 2048] bf16 roped keys, zero-padded halves
    v    [128, 16kb, 8h, 65] bf16, col 64 = ones (softmax denominator)
    att  ring[2] of [128, 17408] bf16  exp(scores^T), causal-trapezoid packed
    y_mb ring[2] of [128, 16qb, 128] bf16  normalized per-pair outputs
    yT   [128, 4, 2048] bf16  transposed y for the out-projection
"""

import os
import sys

import numpy as np

for _p in ("/opt/trn_rl_repo", "/root/.axon_site/_ro/trn_rl_repo"):
    if os.path.isdir(_p) and _p not in sys.path:
        sys.path.append(_p)

import ml_dtypes  # noqa: E402

BF16 = ml_dtypes.bfloat16

B, S, D_MODEL = 4, 2048, 1024
N_HEADS, HEAD_DIM = 16, 64
N_CORES = 8
HG = 2                      # head groups
HPC = N_HEADS // HG         # heads per core = 8
DL = HPC * HEAD_DIM         # local dims per core = 512
SCALE = HEAD_DIM ** -0.5
P = 128
KC = D_MODEL // P           # k chunks in projections = 8
MB = DL // P                # m blocks (head pairs) = 4
NKB = S // P                # 128-row blocks of sequence = 16
NT = S // DL                # 4 sequence tiles of 512
QK_W = 1024                 # scores psum strip width (2 banks)
HH = HEAD_DIM // 2          # 32

# packed causal-trapezoid offsets: att row-block ck covers q in [128*ck, S)
ATT_OFF = [0] * (NKB + 1)
for _ck in range(NKB):
    ATT_OFF[_ck + 1] = ATT_OFF[_ck] + (S - P * _ck)
ATT_TOT = ATT_OFF[NKB]      # 17408

_CACHE = {}


def _interleave(*gens):
    """Round-robin the generators until all are exhausted."""
    alive = list(gens)
    while alive:
        nxt = []
        for g in alive:
            if next(g, "done") != "done":
                nxt.append(g)
        alive = nxt


def _build_bass():
    import concourse.tile as tile
    from concourse import bacc, mybir

    dt = mybir.dt
    nc = bacc.Bacc("TRN2", target_bir_lowering=False, debug=False)

    def din(name, shape, d=dt.bfloat16):
        return nc.dram_tensor(name, shape, d, kind="ExternalInput").ap()

    xT_d = din("xT", [D_MODEL, S])
    wq_d = din("wq", [D_MODEL, DL])
    wk_d = din("wk", [D_MODEL, DL])
    wv_d = din("wv", [D_MODEL, DL])
    wo_d = din("wo", [DL, D_MODEL])
    bq_d = din("bqT", [P, MB], dt.float32)
    bk_d = din("bkT", [P, MB], dt.float32)
    bv_d = din("bv", [1, DL])
    cos_d = din("cosT", [P, S])
    sin_d = din("sinT", [P, S])          # sign-folded (pi-basis)
    negi_d = din("negi", [P, P])         # -1e9 * I
    ltri_d = din("ltri", [P, P])         # strict lower triangular ones
    ident_d = din("ident", [P, P])
    o_d = nc.dram_tensor("o", [S, D_MODEL], dt.bfloat16,
                         kind="ExternalOutput").ap()

    FCopy = mybir.ActivationFunctionType.Copy
    FExp = mybir.ActivationFunctionType.Exp
    Mult = mybir.AluOpType.mult
    Add = mybir.AluOpType.add

    with tile.TileContext(nc) as tc:
        with (
            tc.tile_pool(name="persist", bufs=1) as persist,
            tc.tile_pool(name="ring2", bufs=2) as ring2,
            tc.tile_pool(name="att_pool", bufs=2) as att_pool,
            tc.tile_pool(name="bstage", bufs=2) as bstage,
            # PSUM banks: proj 2 | qk 2x2 | tp 1 | y 1  -> 8 total
            tc.tile_pool(name="proj_ps", bufs=2, space="PSUM") as proj_ps,
            tc.tile_pool(name="qk_ps", bufs=2, space="PSUM") as qk_ps,
            tc.tile_pool(name="tp_pool", bufs=1, space="PSUM") as tp_pool,
            tc.tile_pool(name="y_ps", bufs=1, space="PSUM") as y_ps_pool,
        ):
            # ---------------- persistent SBUF ----------------
            xT_sb = persist.tile([P, KC, S], dt.bfloat16, tag="xT", name="xT")
            wv_sb = persist.tile([P, KC, DL], dt.bfloat16, tag="wv", name="wv")
            wo_sb = persist.tile([P, MB, D_MODEL], dt.bfloat16, tag="wo",
                                 name="wo")
            cos_sb = persist.tile([P, S], dt.bfloat16, tag="cos", name="cos")
            sin_sb = persist.tile([P, S], dt.bfloat16, tag="sin", name="sin")
            v_sb = persist.tile([P, NKB, HPC, HEAD_DIM + 1], dt.bfloat16,
                                tag="v_sb", name="v_sb")
            yT_all = persist.tile([P, MB, S], dt.bfloat16, tag="yT", name="yT")
            bq_sb = persist.tile([P, MB], dt.float32, tag="bq", name="bq")
            bk_sb = persist.tile([P, MB], dt.float32, tag="bk", name="bk")
            bv_sb = persist.tile([1, DL], dt.bfloat16, tag="bv", name="bv")
            ones_sb = persist.tile([1, P], dt.bfloat16, tag="ones", name="ones")
            negi_sb = persist.tile([P, P], dt.bfloat16, tag="negi", name="negi")
            ltri_sb = persist.tile([P, P], dt.bfloat16, tag="ltri", name="ltri")
            ident_sb = persist.tile([P, P], dt.bfloat16, tag="ident",
                                    name="ident")

            nc.vector.memset(ones_sb, 1.0)
            nc.vector.memset(v_sb[:, :, :, HEAD_DIM:HEAD_DIM + 1], 1.0)

            # persistent PSUM scratch: 3-slot AV accumulator + transpose tile
            y_all_ps = y_ps_pool.tile([P, 3, HEAD_DIM + 2], dt.float32,
                                      tag="yp", name="yp")
            tp_ps = tp_pool.tile([P, 4 * P], dt.bfloat16, tag="tp",
                                 name="tp")

            # ---------------- input DMAs (sync queue) ----------------
            def dma_w(dst, src):
                for kc in range(KC):
                    nc.sync.dma_start(out=dst[:, kc, :],
                                      in_=src[kc * P:(kc + 1) * P, :])

            def dma_wm(dst, src, m):
                # one m-column-block [P, KC, P] of a projection weight
                for kc in range(KC):
                    nc.sync.dma_start(
                        out=dst[:, kc, :],
                        in_=src[kc * P:(kc + 1) * P, m * P:(m + 1) * P])

            def dma_xt(t):
                for kc in range(KC):
                    nc.sync.dma_start(
                        out=xT_sb[:, kc, t * DL:(t + 1) * DL],
                        in_=xT_d[kc * P:(kc + 1) * P, t * DL:(t + 1) * DL])

            dma_xt(0)
            nc.sync.dma_start(out=bq_sb, in_=bq_d)
            nc.sync.dma_start(out=bk_sb, in_=bk_d)
            nc.sync.dma_start(out=cos_sb, in_=cos_d)
            nc.sync.dma_start(out=sin_sb, in_=sin_d)
            dma_xt(1)
            nc.sync.dma_start(out=negi_sb, in_=negi_d)
            nc.sync.dma_start(out=ltri_sb, in_=ltri_d)
            nc.sync.dma_start(out=ident_sb, in_=ident_d)
            dma_xt(2)
            dma_w(wv_sb, wv_d)
            nc.sync.dma_start(out=bv_sb, in_=bv_d)
            dma_xt(3)
            nc.sync.dma_start(
                out=wo_sb, in_=wo_d.rearrange("(m p) n -> p m n", p=P))

            # ---------------- projection + RoPE ----------------
            qT_ring = {}
            kT_ring = {}

            def emit_proj_round(m):
                # one step per (t, which) for interleaving; order q,k per t
                qT = ring2.tile([P, S], dt.bfloat16, tag="qT",
                                name=f"qT{m}")
                kT = ring2.tile([P, 2, S], dt.bfloat16, tag="kT",
                                name=f"kT{m}")
                wqm = ring2.tile([P, KC, P], dt.bfloat16, tag="wqm",
                                 name=f"wqm{m}")
                wkm = ring2.tile([P, KC, P], dt.bfloat16, tag="wkm",
                                 name=f"wkm{m}")
                dma_wm(wqm, wq_d, m)
                dma_wm(wkm, wk_d, m)
                qT_ring[m] = qT
                kT_ring[m] = kT
                nc.vector.memset(kT, 0.0)
                for t in range(NT):
                    ts = slice(t * DL, (t + 1) * DL)
                    for nm in ("q", "k"):
                        w_sb = wqm if nm == "q" else wkm
                        b_sb = bq_sb if nm == "q" else bk_sb
                        ps = proj_ps.tile([P, DL], dt.float32, tag="proj",
                                          name="ps_proj")
                        for kc in range(KC):
                            nc.tensor.matmul(
                                ps, lhsT=w_sb[:, kc, :],
                                rhs=xT_sb[:, kc, ts],
                                start=(kc == 0), stop=(kc == KC - 1))
                        # bias add + copy out of PSUM (DVE)
                        raw = bstage.tile([P, DL], dt.bfloat16, tag="raw",
                                          name="raw")
                        nc.vector.tensor_scalar(
                            raw, ps, b_sb[:, m:m + 1], None, Add)
                        # rotate-half in the permuted basis: swap the
                        # 32-row halves of each head (sign lives in sinT)
                        qsw = bstage.tile([P, DL], dt.bfloat16, tag="qsw",
                                          name="qsw")
                        for hh in range(2):
                            o32 = hh * HEAD_DIM
                            nc.sync.dma_start(
                                out=qsw[o32:o32 + HH, :],
                                in_=raw[o32 + HH:o32 + HEAD_DIM, :])
                            nc.sync.dma_start(
                                out=qsw[o32 + HH:o32 + HEAD_DIM, :],
                                in_=raw[o32:o32 + HH, :])
                        # raw *= cos (DVE, in place), qsw *= sin (Pool)
                        nc.vector.tensor_mul(raw, raw, cos_sb[:, ts])
                        nc.gpsimd.tensor_mul(qsw, qsw, sin_sb[:, ts])
                        if nm == "q":
                            nc.vector.tensor_add(qT[:, ts], raw, qsw)
                        else:
                            for hh in range(2):
                                po = hh * HEAD_DIM
                                nc.vector.tensor_add(
                                    kT[po:po + HEAD_DIM, hh, ts],
                                    raw[po:po + HEAD_DIM, :],
                                    qsw[po:po + HEAD_DIM, :])
                        yield

            def emit_v(kb):
                # v projection: natural [seq, dims] layout + ones cols
                ps = proj_ps.tile([P, DL], dt.float32, tag="proj",
                                  name="ps_v")
                for kc in range(KC):
                    nc.tensor.matmul(
                        ps, lhsT=xT_sb[:, kc, kb * P:(kb + 1) * P],
                        rhs=wv_sb[:, kc, :],
                        start=(kc == 0), stop=False)
                nc.tensor.matmul(
                    ps, lhsT=ones_sb, rhs=bv_sb,
                    start=False, stop=True)
                nc.scalar.activation(
                    out=v_sb[:, kb, :, 0:HEAD_DIM],
                    in_=ps.rearrange("p (h d) -> p h d", h=HPC), func=FCopy)

            def emit_v_steps():
                for kb in range(NKB):
                    emit_v(kb)
                    yield

            # ---------------- attention ----------------
            def emit_c1_steps(h, att):
                # per ck: QK^T (+causal mask fold) then exp into att
                m = h // 2
                kh = kT_ring[m][:, h % 2, :]
                qh = qT_ring[m]
                for ck in range(NKB):
                    w = S - ck * P
                    base = ck * P
                    off = ATT_OFF[ck]
                    for s0 in range(0, w, QK_W):
                        sw = min(QK_W, w - s0)
                        ps = qk_ps.tile([P, QK_W], dt.float32, tag="qk",
                                        name="ps_qk")
                        for u0 in range(0, sw, DL):
                            uw = min(DL, sw - u0)
                            first = (s0 == 0 and u0 == 0)
                            nc.tensor.matmul(
                                ps[:, u0:u0 + uw],
                                lhsT=kh[:, ck * P:(ck + 1) * P],
                                rhs=qh[:, base + s0 + u0:base + s0 + u0 + uw],
                                start=True, stop=not first,
                                skip_group_check=True)
                            if first:
                                # fold the causal mask of the diagonal block
                                # into the accumulation: += -1e9 * [k > q]
                                nc.tensor.matmul(
                                    ps[:, 0:P], lhsT=negi_sb, rhs=ltri_sb,
                                    start=False, stop=True,
                                    skip_group_check=True)
                        nc.scalar.activation(
                            out=att[:, off + s0:off + s0 + sw],
                            in_=ps[:, 0:sw], func=FExp, scale=SCALE)
                    yield

            c2_slot = [0]

            def emit_c2_steps(h, att, y_mb):
                # per qb (descending): AV chain + divide-normalize
                po = (h % 2) * HEAD_DIM
                for qb in range(NKB - 1, -1, -1):
                    ys = y_all_ps[:, c2_slot[0] % 3, 0:HEAD_DIM + 1]
                    c2_slot[0] += 1
                    for ck in range(qb + 1):
                        a0 = ATT_OFF[ck] + (qb - ck) * P
                        nc.tensor.matmul(
                            ys, lhsT=att[:, a0:a0 + P],
                            rhs=v_sb[:, ck, h, :],
                            start=(ck == 0), stop=(ck == qb))
                    r = bstage.tile([P, 1], dt.float32, tag="r", name="r")
                    nc.vector.reciprocal(r, ys[:, HEAD_DIM:HEAD_DIM + 1])
                    nc.vector.tensor_scalar(
                        y_mb[:, qb, po:po + HEAD_DIM],
                        ys[:, 0:HEAD_DIM], r, None, Mult)
                    yield

            def emit_ytrans_steps(m, y_mb):
                for q4 in range(NKB // 4):
                    tp = tp_ps
                    for j in range(4):
                        nc.tensor.transpose(
                            tp[:, j * P:(j + 1) * P],
                            y_mb[:, q4 * 4 + j, :], ident_sb)
                    nc.vector.tensor_copy(
                        out=yT_all[:, m, q4 * 4 * P:(q4 + 1) * 4 * P],
                        in_=tp)
                    yield

            def emit_outproj_steps():
                for qb in range(NKB):
                    for t in range(2):
                        ps = proj_ps.tile([P, DL], dt.float32, tag="proj",
                                          name="ps_o")
                        for m in range(MB):
                            nc.tensor.matmul(
                                ps, lhsT=yT_all[:, m, qb * P:(qb + 1) * P],
                                rhs=wo_sb[:, m, t * DL:(t + 1) * DL],
                                start=(m == 0), stop=(m == MB - 1))
                        ob = bstage.tile([P, DL], dt.bfloat16, tag="ob",
                                         name="ob")
                        nc.scalar.activation(out=ob, in_=ps, func=FCopy)
                        nc.sync.dma_start(
                            out=o_d[qb * P:(qb + 1) * P,
                                    t * DL:(t + 1) * DL],
                            in_=ob)
                    yield

            # ---------------- schedule ----------------
            att_t = {}
            y_mb_t = {}

            def new_att(h):
                att_t[h] = att_pool.tile([P, ATT_TOT], dt.bfloat16, tag="att",
                                         name=f"att{h}")
                return att_t[h]

            def new_ymb(m):
                y_mb_t[m] = ring2.tile([P, NKB, P], dt.bfloat16, tag="y_mb",
                                       name=f"y_mb{m}")
                return y_mb_t[m]

            # round 0: project pair 0, then v interleaved with head-0 QK/exp
            for _ in emit_proj_round(0):
                pass
            new_ymb(0)
            _interleave(emit_v_steps(), emit_c1_steps(0, new_att(0)))
            _interleave(emit_c2_steps(0, att_t[0], y_mb_t[0]),
                        emit_c1_steps(1, new_att(1)))
            # rounds 1..3
            for r in range(1, MB):
                _interleave(emit_c2_steps(2 * r - 1, att_t[2 * r - 1],
                                          y_mb_t[r - 1]),
                            emit_proj_round(r))
                new_ymb(r)
                _interleave(emit_c1_steps(2 * r, new_att(2 * r)),
                            emit_ytrans_steps(r - 1, y_mb_t[r - 1]))
                _interleave(emit_c2_steps(2 * r, att_t[2 * r], y_mb_t[r]),
                            emit_c1_steps(2 * r + 1, new_att(2 * r + 1)))
            # tail: last head's AV, last transpose, out-projection
            for _ in emit_c2_steps(HPC - 1, att_t[HPC - 1], y_mb_t[MB - 1]):
                pass
            for _ in emit_ytrans_steps(MB - 1, y_mb_t[MB - 1]):
                pass
            for _ in emit_outproj_steps():
                pass

    nc.compile()
    return nc


def _perm64():
    # pi: permuted-basis index j -> original head dim (evens then odds)
    return np.concatenate([np.arange(0, HEAD_DIM, 2), np.arange(1, HEAD_DIM, 2)])


def _host_tables():
    pos = np.arange(S, dtype=np.float32)
    freq = np.arange(0, HEAD_DIM, 2, dtype=np.float32) / HEAD_DIM
    inv_freq = 1.0 / (10000.0 ** freq)                  # [32]
    ang = np.outer(inv_freq, pos)                       # [32, S]
    cos1 = np.cos(ang)
    sin1 = np.sin(ang)
    # pi-basis per-head tables [64, S]: rows 0..31 evens, 32..63 odds
    cosh = np.concatenate([cos1, cos1], axis=0)
    sinh = np.concatenate([-sin1, sin1], axis=0)        # sign folded in
    cosT = np.tile(cosh, (2, 1))                        # [128, S] head pair
    sinT = np.tile(sinh, (2, 1))
    negi = -1e9 * np.eye(P, dtype=np.float32)
    ltri = np.tril(np.ones((P, P), np.float32), k=-1)   # strict lower: r > c
    ident = np.eye(P, dtype=np.float32)
    return (cosT.astype(BF16), sinT.astype(BF16),
            negi.astype(BF16), ltri.astype(BF16), ident.astype(BF16))


def kernel(x, Wq, bq, Wk, bk, Wv, bv, Wo, bo):
    from concourse.bass_utils import run_bass_kernel_spmd

    x = np.asarray(x, np.float32)
    Wq, Wk, Wv, Wo = (np.asarray(a, np.float32) for a in (Wq, Wk, Wv, Wo))
    bq, bk, bv, bo = (np.asarray(a, np.float32) for a in (bq, bk, bv, bo))

    if "nc" not in _CACHE:
        _CACHE["nc"] = _build_bass()
    nc = _CACHE["nc"]

    cosT, sinT, negi, ltri, ident = _host_tables()
    consts = {"cosT": cosT, "sinT": sinT, "negi": negi, "ltri": ltri,
              "ident": ident}

    # pi-basis permutation of q/k projection columns (per head)
    pi = _perm64()
    colperm = np.concatenate([h * HEAD_DIM + pi for h in range(N_HEADS)])
    Wq_p = Wq[:, colperm]
    Wk_p = Wk[:, colperm]
    bq_p = bq[colperm]
    bk_p = bk[colperm]

    xTs = [np.ascontiguousarray(x[b].T).astype(BF16) for b in range(B)]
    in_maps = []
    for c in range(N_CORES):
        b, g = c // HG, c % HG
        sl = slice(g * DL, (g + 1) * DL)
        in_maps.append({
            "xT": xTs[b],
            "wq": np.ascontiguousarray(Wq_p[:, sl]).astype(BF16),
            "wk": np.ascontiguousarray(Wk_p[:, sl]).astype(BF16),
            "wv": np.ascontiguousarray(Wv[:, sl]).astype(BF16),
            "wo": np.ascontiguousarray(Wo[sl, :]).astype(BF16),
            "bqT": np.ascontiguousarray(
                bq_p[sl].reshape(MB, P).T).astype(np.float32),
            "bkT": np.ascontiguousarray(
                bk_p[sl].reshape(MB, P).T).astype(np.float32),
            "bv": bv[sl].reshape(1, DL).astype(BF16),
            **consts,
        })

    res = run_bass_kernel_spmd(nc, in_maps, core_ids=list(range(N_CORES)))
    _CACHE["last_result"] = res
    out = np.empty((B, S, D_MODEL), np.float32)
    for b in range(B):
        out[b] = (res.results[HG * b]["o"].astype(np.float32) +
                  res.results[HG * b + 1]["o"].astype(np.float32))
    out += bo.astype(np.float32)
    return out


# revision 26
# speedup vs baseline: 1.1671x; 1.1050x over previous
"""Multi-head self-attention (RoPE, causal) Trainium2 Bass kernel.

Sharding: 8 cores = 4 batches x 2 head-groups (8 heads each).
Per core the device kernel computes, for its batch b and head-group g:
    q/k/v = x_b @ W*[:, g] (+bias), RoPE on q/k, causal softmax attention,
    partial out-projection y @ Wo[g]  -> [2048, 1024] (f32).
Host sums the two head-group partials per batch and adds bo.

Pipeline layout (single TileContext, engines overlapped end to end):
    round m = 0..3:  project+RoPE q/k for head pair m (PE+DVE+Pool),
    then attention for heads 2m, 2m+1 (QK on PE -> exp on Act -> AV on PE),
    software-pipelined so the Act engine's exp stream (the serial resource:
    only the Act engine has activation tables) runs concurrently with the
    next group's projections and the previous head's AV matmuls.

The causal mask is folded into the QK PSUM accumulation as an extra matmul
(-1e9*I)^T @ strict_lower_ones on the diagonal block, so exp() gives exact
zeros in the masked region and no separate mask multiply is needed.

RoPE runs in a per-head permuted basis (evens then odds) prepared on the
host by permuting Wq/Wk columns: rotate-half becomes a swap of contiguous
32-partition halves (SBUF->SBUF DMAs) with the signs folded into the sin
table.  Scores are invariant to the shared q/k basis permutation.

Device layouts (per core):
    xT   [128, 8, 2048] bf16  x_b transposed (host-prepped)
    qT   ring[2] of [128, 2048] bf16   roped queries, pair dims on partitions
    kT   ring[2] of [128, 2, 2048] bf16 roped keys, zero-padded halves
    v    [128, 16kb, 8h, 65] bf16, col 64 = ones (softmax denominator)
    att  ring[2] of [128, 17408] bf16  exp(scores^T), causal-trapezoid packed
    y_mb ring[2] of [128, 16qb, 128] bf16  normalized per-pair outputs
    yT   [128, 4, 2048] bf16  transposed y for the out-projection
"""

import os
import sys

import numpy as np

for _p in ("/opt/trn_rl_repo", "/root/.axon_site/_ro/trn_rl_repo"):
    if os.path.isdir(_p) and _p not in sys.path:
        sys.path.append(_p)

import ml_dtypes  # noqa: E402

BF16 = ml_dtypes.bfloat16

B, S, D_MODEL = 4, 2048, 1024
N_HEADS, HEAD_DIM = 16, 64
N_CORES = 8
HG = 2                      # head groups
HPC = N_HEADS // HG         # heads per core = 8
DL = HPC * HEAD_DIM         # local dims per core = 512
SCALE = HEAD_DIM ** -0.5
P = 128
KC = D_MODEL // P           # k chunks in projections = 8
MB = DL // P                # m blocks (head pairs) = 4
NKB = S // P                # 128-row blocks of sequence = 16
NT = S // DL                # 4 sequence tiles of 512
QK_W = 1024                 # scores psum strip width (2 banks)
HH = HEAD_DIM // 2          # 32

# packed causal-trapezoid offsets: att row-block ck covers q in [128*ck, S)
ATT_OFF = [0] * (NKB + 1)
for _ck in range(NKB):
    ATT_OFF[_ck + 1] = ATT_OFF[_ck] + (S - P * _ck)
ATT_TOT = ATT_OFF[NKB]      # 17408

_CACHE = {}


def _interleave(*gens):
    """Round-robin the generators until all are exhausted."""
    alive = list(gens)
    while alive:
        nxt = []
        for g in alive:
            if next(g, "done") != "done":
                nxt.append(g)
        alive = nxt


def _build_bass():
    import concourse.tile as tile
    from concourse import bacc, mybir

    dt = mybir.dt
    nc = bacc.Bacc("TRN2", target_bir_lowering=False, debug=False)

    def din(name, shape, d=dt.bfloat16):
        return nc.dram_tensor(name, shape, d, kind="ExternalInput").ap()

    xT_d = din("xT", [D_MODEL, S])
    wq_d = din("wq", [D_MODEL, DL])
    wk_d = din("wk", [D_MODEL, DL])
    wv_d = din("wv", [D_MODEL, DL])
    wo_d = din("wo", [DL, D_MODEL])
    bq_d = din("bqT", [P, MB], dt.float32)
    bk_d = din("bkT", [P, MB], dt.float32)
    bv_d = din("bv", [1, DL])
    cos_d = din("cosT", [P, S])
    sin_d = din("sinT", [P, S])          # sign-folded (pi-basis)
    negi_d = din("negi", [P, P])         # -1e9 * I
    ltri_d = din("ltri", [P, P])         # strict lower triangular ones
    ident_d = din("ident", [P, P])
    o_d = nc.dram_tensor("o", [S, D_MODEL], dt.bfloat16,
                         kind="ExternalOutput").ap()

    FCopy = mybir.ActivationFunctionType.Copy
    FExp = mybir.ActivationFunctionType.Exp
    Mult = mybir.AluOpType.mult
    Add = mybir.AluOpType.add

    with tile.TileContext(nc) as tc:
        with (
            tc.tile_pool(name="persist", bufs=1) as persist,
            tc.tile_pool(name="ring2", bufs=2) as ring2,
            tc.tile_pool(name="att_pool", bufs=2) as att_pool,
            tc.tile_pool(name="bstage", bufs=2) as bstage,
            # PSUM banks: proj 2 | qk 2x2 | tp 1 | y 1  -> 8 total
            tc.tile_pool(name="proj_ps", bufs=2, space="PSUM") as proj_ps,
            tc.tile_pool(name="qk_ps", bufs=2, space="PSUM") as qk_ps,
            tc.tile_pool(name="tp_pool", bufs=1, space="PSUM") as tp_pool,
            tc.tile_pool(name="y_ps", bufs=1, space="PSUM") as y_ps_pool,
        ):
            # ---------------- persistent SBUF ----------------
            xT_sb = persist.tile([P, KC, S], dt.bfloat16, tag="xT", name="xT")
            wv_sb = persist.tile([P, KC, DL], dt.bfloat16, tag="wv", name="wv")
            wo_sb = persist.tile([P, MB, D_MODEL], dt.bfloat16, tag="wo",
                                 name="wo")
            cos_sb = persist.tile([P, S], dt.bfloat16, tag="cos", name="cos")
            sin_sb = persist.tile([P, S], dt.bfloat16, tag="sin", name="sin")
            v_sb = persist.tile([P, NKB, HPC, HEAD_DIM + 1], dt.bfloat16,
                                tag="v_sb", name="v_sb")
            yT_all = persist.tile([P, MB, S], dt.bfloat16, tag="yT", name="yT")
            bq_sb = persist.tile([P, MB], dt.float32, tag="bq", name="bq")
            bk_sb = persist.tile([P, MB], dt.float32, tag="bk", name="bk")
            bv_sb = persist.tile([1, DL], dt.bfloat16, tag="bv", name="bv")
            ones_sb = persist.tile([1, P], dt.bfloat16, tag="ones", name="ones")
            negi_sb = persist.tile([P, P], dt.bfloat16, tag="negi", name="negi")
            ltri_sb = persist.tile([P, P], dt.bfloat16, tag="ltri", name="ltri")
            ident_sb = persist.tile([P, P], dt.bfloat16, tag="ident",
                                    name="ident")

            nc.vector.memset(ones_sb, 1.0)
            nc.vector.memset(v_sb[:, :, :, HEAD_DIM:HEAD_DIM + 1], 1.0)

            # persistent PSUM scratch: 3-slot AV accumulator + transpose tile
            y_all_ps = y_ps_pool.tile([P, 3, HEAD_DIM + 2], dt.float32,
                                      tag="yp", name="yp")
            tp_ps = tp_pool.tile([P, 4 * P], dt.bfloat16, tag="tp",
                                 name="tp")

            # ---------------- input DMAs (sync queue, batched) ----------------
            wqv_d = wq_d.rearrange("(kc p) n -> p kc n", p=P)
            wkv_d = wk_d.rearrange("(kc p) n -> p kc n", p=P)
            xTv_d = xT_d.rearrange("(kc p) s -> p kc s", p=P)

            def dma_xt(t):
                ts = slice(t * DL, (t + 1) * DL)
                nc.sync.dma_start(out=xT_sb[:, :, ts], in_=xTv_d[:, :, ts])

            wqm_t = {}
            wkm_t = {}

            def alloc_weights(m):
                # m-column-block [P, KC, P] of wq/wk, one trigger each
                if m >= MB:
                    return
                wqm = ring2.tile([P, KC, P], dt.bfloat16, tag="wqm",
                                 name=f"wqm{m}")
                wkm = ring2.tile([P, KC, P], dt.bfloat16, tag="wkm",
                                 name=f"wkm{m}")
                ms = slice(m * P, (m + 1) * P)
                nc.sync.dma_start(out=wqm, in_=wqv_d[:, :, ms])
                nc.sync.dma_start(out=wkm, in_=wkv_d[:, :, ms])
                wqm_t[m] = wqm
                wkm_t[m] = wkm

            alloc_weights(0)
            dma_xt(0)
            nc.sync.dma_start(out=bq_sb, in_=bq_d)
            nc.sync.dma_start(out=bk_sb, in_=bk_d)
            nc.sync.dma_start(out=cos_sb, in_=cos_d)
            nc.sync.dma_start(out=sin_sb, in_=sin_d)
            dma_xt(1)
            nc.sync.dma_start(out=negi_sb, in_=negi_d)
            nc.sync.dma_start(out=ltri_sb, in_=ltri_d)
            nc.sync.dma_start(out=ident_sb, in_=ident_d)
            dma_xt(2)
            nc.sync.dma_start(
                out=wv_sb, in_=wv_d.rearrange("(kc p) n -> p kc n", p=P))
            nc.sync.dma_start(out=bv_sb, in_=bv_d)
            dma_xt(3)
            nc.sync.dma_start(
                out=wo_sb, in_=wo_d.rearrange("(m p) n -> p m n", p=P))

            # ---------------- projection + RoPE ----------------
            qT_ring = {}
            kT_ring = {}

            def emit_proj_round(m):
                # one step per (t, which) for interleaving; order q,k per t
                qT = ring2.tile([P, S], dt.bfloat16, tag="qT",
                                name=f"qT{m}")
                kT = ring2.tile([P, 2, S], dt.bfloat16, tag="kT",
                                name=f"kT{m}")
                wqm, wkm = wqm_t[m], wkm_t[m]
                qT_ring[m] = qT
                kT_ring[m] = kT
                nc.vector.memset(kT, 0.0)
                for t in range(NT):
                    ts = slice(t * DL, (t + 1) * DL)
                    for nm in ("q", "k"):
                        w_sb = wqm if nm == "q" else wkm
                        b_sb = bq_sb if nm == "q" else bk_sb
                        ps = proj_ps.tile([P, DL], dt.float32, tag="proj",
                                          name="ps_proj")
                        for kc in range(KC):
                            nc.tensor.matmul(
                                ps, lhsT=w_sb[:, kc, :],
                                rhs=xT_sb[:, kc, ts],
                                start=(kc == 0), stop=(kc == KC - 1))
                        # bias add + copy out of PSUM (DVE)
                        raw = bstage.tile([P, DL], dt.bfloat16, tag="raw",
                                          name="raw")
                        nc.vector.tensor_scalar(
                            raw, ps, b_sb[:, m:m + 1], None, Add)
                        # rotate-half in the permuted basis: swap the
                        # 32-row halves of each head (sign lives in sinT)
                        qsw = bstage.tile([P, DL], dt.bfloat16, tag="qsw",
                                          name="qsw")
                        for hh in range(2):
                            o32 = hh * HEAD_DIM
                            nc.gpsimd.dma_start(
                                out=qsw[o32:o32 + HH, :],
                                in_=raw[o32 + HH:o32 + HEAD_DIM, :])
                            nc.gpsimd.dma_start(
                                out=qsw[o32 + HH:o32 + HEAD_DIM, :],
                                in_=raw[o32:o32 + HH, :])
                        # raw *= cos (DVE, in place), qsw *= sin (Pool)
                        nc.vector.tensor_mul(raw, raw, cos_sb[:, ts])
                        nc.gpsimd.tensor_mul(qsw, qsw, sin_sb[:, ts])
                        if nm == "q":
                            nc.vector.tensor_add(qT[:, ts], raw, qsw)
                        else:
                            for hh in range(2):
                                po = hh * HEAD_DIM
                                nc.vector.tensor_add(
                                    kT[po:po + HEAD_DIM, hh, ts],
                                    raw[po:po + HEAD_DIM, :],
                                    qsw[po:po + HEAD_DIM, :])
                        yield
                alloc_weights(m + 1)

            def emit_v(kb):
                # v projection: natural [seq, dims] layout + ones cols
                ps = proj_ps.tile([P, DL], dt.float32, tag="proj",
                                  name="ps_v")
                for kc in range(KC):
                    nc.tensor.matmul(
                        ps, lhsT=xT_sb[:, kc, kb * P:(kb + 1) * P],
                        rhs=wv_sb[:, kc, :],
                        start=(kc == 0), stop=False)
                nc.tensor.matmul(
                    ps, lhsT=ones_sb, rhs=bv_sb,
                    start=False, stop=True)
                nc.vector.tensor_copy(
                    out=v_sb[:, kb, :, 0:HEAD_DIM],
                    in_=ps.rearrange("p (h d) -> p h d", h=HPC))

            def emit_v_steps():
                for kb in range(NKB):
                    emit_v(kb)
                    yield

            # ---------------- attention ----------------
            def emit_c1_steps(h, att):
                # per ck: QK^T (+causal mask fold) then exp into att
                m = h // 2
                kh = kT_ring[m][:, h % 2, :]
                qh = qT_ring[m]
                for ck in range(NKB):
                    w = S - ck * P
                    base = ck * P
                    off = ATT_OFF[ck]
                    for s0 in range(0, w, QK_W):
                        sw = min(QK_W, w - s0)
                        ps = qk_ps.tile([P, QK_W], dt.float32, tag="qk",
                                        name="ps_qk")
                        for u0 in range(0, sw, DL):
                            uw = min(DL, sw - u0)
                            first = (s0 == 0 and u0 == 0)
                            nc.tensor.matmul(
                                ps[:, u0:u0 + uw],
                                lhsT=kh[:, ck * P:(ck + 1) * P],
                                rhs=qh[:, base + s0 + u0:base + s0 + u0 + uw],
                                start=True, stop=not first,
                                skip_group_check=True)
                            if first:
                                # fold the causal mask of the diagonal block
                                # into the accumulation: += -1e9 * [k > q]
                                nc.tensor.matmul(
                                    ps[:, 0:P], lhsT=negi_sb, rhs=ltri_sb,
                                    start=False, stop=True,
                                    skip_group_check=True)
                        nc.scalar.activation(
                            out=att[:, off + s0:off + s0 + sw],
                            in_=ps[:, 0:sw], func=FExp, scale=SCALE)
                    yield

            c2_slot = [0]

            def emit_c2_steps(h, att, y_mb):
                # per qb (ascending: AV for qb only needs v[0..qb])
                po = (h % 2) * HEAD_DIM
                for qb in range(NKB):
                    ys = y_all_ps[:, c2_slot[0] % 3, 0:HEAD_DIM + 1]
                    c2_slot[0] += 1
                    for ck in range(qb + 1):
                        a0 = ATT_OFF[ck] + (qb - ck) * P
                        nc.tensor.matmul(
                            ys, lhsT=att[:, a0:a0 + P],
                            rhs=v_sb[:, ck, h, :],
                            start=(ck == 0), stop=(ck == qb))
                    r = bstage.tile([P, 1], dt.float32, tag="r", name="r")
                    nc.vector.reciprocal(r, ys[:, HEAD_DIM:HEAD_DIM + 1])
                    nc.vector.tensor_scalar(
                        y_mb[:, qb, po:po + HEAD_DIM],
                        ys[:, 0:HEAD_DIM], r, None, Mult)
                    yield

            def emit_ytrans_steps(m, y_mb):
                for q4 in range(NKB // 4):
                    tp = tp_ps
                    for j in range(4):
                        nc.tensor.transpose(
                            tp[:, j * P:(j + 1) * P],
                            y_mb[:, q4 * 4 + j, :], ident_sb)
                    nc.vector.tensor_copy(
                        out=yT_all[:, m, q4 * 4 * P:(q4 + 1) * 4 * P],
                        in_=tp)
                    yield

            def emit_outproj(qb):
                for t in range(2):
                    ps = proj_ps.tile([P, DL], dt.float32, tag="proj",
                                      name="ps_o")
                    for m in range(MB):
                        nc.tensor.matmul(
                            ps, lhsT=yT_all[:, m, qb * P:(qb + 1) * P],
                            rhs=wo_sb[:, m, t * DL:(t + 1) * DL],
                            start=(m == 0), stop=(m == MB - 1))
                    ob = bstage.tile([P, DL], dt.bfloat16, tag="ob",
                                     name="ob")
                    nc.scalar.activation(out=ob, in_=ps, func=FCopy)
                    nc.sync.dma_start(
                        out=o_d[qb * P:(qb + 1) * P, t * DL:(t + 1) * DL],
                        in_=ob)

            # ---------------- schedule ----------------
            att_t = {}
            y_mb_t = {}

            def new_att(h):
                att_t[h] = att_pool.tile([P, ATT_TOT], dt.bfloat16, tag="att",
                                         name=f"att{h}")
                return att_t[h]

            def new_ymb(m):
                y_mb_t[m] = ring2.tile([P, NKB, P], dt.bfloat16, tag="y_mb",
                                       name=f"y_mb{m}")
                return y_mb_t[m]

            # round 0: project pair 0, then v interleaved with head-0 QK/exp
            for _ in emit_proj_round(0):
                pass
            new_ymb(0)
            _interleave(emit_v_steps(), emit_c1_steps(0, new_att(0)))
            _interleave(emit_c2_steps(0, att_t[0], y_mb_t[0]),
                        emit_c1_steps(1, new_att(1)))
            # rounds 1..3
            for r in range(1, MB):
                _interleave(emit_c2_steps(2 * r - 1, att_t[2 * r - 1],
                                          y_mb_t[r - 1]),
                            emit_proj_round(r))
                new_ymb(r)
                _interleave(emit_c1_steps(2 * r, new_att(2 * r)),
                            emit_ytrans_steps(r - 1, y_mb_t[r - 1]))
                _interleave(emit_c2_steps(2 * r, att_t[2 * r], y_mb_t[r]),
                            emit_c1_steps(2 * r + 1, new_att(2 * r + 1)))
            # tail: last head's AV interleaved with last transpose and the
            # out-projection (ytrans group g is ready after c2 step 4g+3)
            c2_tail = emit_c2_steps(HPC - 1, att_t[HPC - 1], y_mb_t[MB - 1])
            yt_tail = emit_ytrans_steps(MB - 1, y_mb_t[MB - 1])
            for g in range(4):
                for _ in range(4):
                    next(c2_tail, None)
                next(yt_tail, None)
                for qb in range(4 * g, 4 * g + 4):
                    emit_outproj(qb)

    nc.compile()
    return nc


def _perm64():
    # pi: permuted-basis index j -> original head dim (evens then odds)
    return np.concatenate([np.arange(0, HEAD_DIM, 2), np.arange(1, HEAD_DIM, 2)])


def _host_tables():
    pos = np.arange(S, dtype=np.float32)
    freq = np.arange(0, HEAD_DIM, 2, dtype=np.float32) / HEAD_DIM
    inv_freq = 1.0 / (10000.0 ** freq)                  # [32]
    ang = np.outer(inv_freq, pos)                       # [32, S]
    cos1 = np.cos(ang)
    sin1 = np.sin(ang)
    # pi-basis per-head tables [64, S]: rows 0..31 evens, 32..63 odds
    cosh = np.concatenate([cos1, cos1], axis=0)
    sinh = np.concatenate([-sin1, sin1], axis=0)        # sign folded in
    cosT = np.tile(cosh, (2, 1))                        # [128, S] head pair
    sinT = np.tile(sinh, (2, 1))
    negi = -1e9 * np.eye(P, dtype=np.float32)
    ltri = np.tril(np.ones((P, P), np.float32), k=-1)   # strict lower: r > c
    ident = np.eye(P, dtype=np.float32)
    return (cosT.astype(BF16), sinT.astype(BF16),
            negi.astype(BF16), ltri.astype(BF16), ident.astype(BF16))


def kernel(x, Wq, bq, Wk, bk, Wv, bv, Wo, bo):
    from concourse.bass_utils import run_bass_kernel_spmd

    x = np.asarray(x, np.float32)
    Wq, Wk, Wv, Wo = (np.asarray(a, np.float32) for a in (Wq, Wk, Wv, Wo))
    bq, bk, bv, bo = (np.asarray(a, np.float32) for a in (bq, bk, bv, bo))

    if "nc" not in _CACHE:
        _CACHE["nc"] = _build_bass()
    nc = _CACHE["nc"]

    cosT, sinT, negi, ltri, ident = _host_tables()
    consts = {"cosT": cosT, "sinT": sinT, "negi": negi, "ltri": ltri,
              "ident": ident}

    # pi-basis permutation of q/k projection columns (per head)
    pi = _perm64()
    colperm = np.concatenate([h * HEAD_DIM + pi for h in range(N_HEADS)])
    Wq_p = Wq[:, colperm]
    Wk_p = Wk[:, colperm]
    bq_p = bq[colperm]
    bk_p = bk[colperm]

    xTs = [np.ascontiguousarray(x[b].T).astype(BF16) for b in range(B)]
    in_maps = []
    for c in range(N_CORES):
        b, g = c // HG, c % HG
        sl = slice(g * DL, (g + 1) * DL)
        in_maps.append({
            "xT": xTs[b],
            "wq": np.ascontiguousarray(Wq_p[:, sl]).astype(BF16),
            "wk": np.ascontiguousarray(Wk_p[:, sl]).astype(BF16),
            "wv": np.ascontiguousarray(Wv[:, sl]).astype(BF16),
            "wo": np.ascontiguousarray(Wo[sl, :]).astype(BF16),
            "bqT": np.ascontiguousarray(
                bq_p[sl].reshape(MB, P).T).astype(np.float32),
            "bkT": np.ascontiguousarray(
                bk_p[sl].reshape(MB, P).T).astype(np.float32),
            "bv": bv[sl].reshape(1, DL).astype(BF16),
            **consts,
        })

    res = run_bass_kernel_spmd(nc, in_maps, core_ids=list(range(N_CORES)))
    _CACHE["last_result"] = res
    out = np.empty((B, S, D_MODEL), np.float32)
    for b in range(B):
        out[b] = (res.results[HG * b]["o"].astype(np.float32) +
                  res.results[HG * b + 1]["o"].astype(np.float32))
    out += bo.astype(np.float32)
    return out
